# revision 1
# baseline (speedup 1.0000x reference)
"""Banded causal self-attention (sparse_attention) for 8 trn2 NeuronCores.

Sharding: tensor-parallel over head groups (4 groups x 4 heads of dim 64)
x data-parallel over batch (2). Core c handles batch c//4, head group c%4.
Each core computes a partial output projection; the host sums the 4 group
partials per batch.

Layout: x is transposed on the host so every matmul on device uses natural
(pre-transposed) operands:
  qkT[512, T]   = W_qk.T @ x.T      (lhsT = W_qk natural, rhs = xT)
  v[T, 256]     = x @ W_v           (lhsT = xT natural,   rhs = W_v)
  scoresT[tk,tq]  computed as lhsT=kT rhs=qT  (both slices of qkT)
  yT+sums       = lhsT=[v|1] rhs=exp(scoresT)  (sums row = softmax denom)
  out[T, C]     = lhsT=yTpair rhs=W_p pair rows (K=128, 2 pairs)
Softmax skips max-subtraction (scores ~ N(0,1) after 1/8 scale; exp is safe
in fp32), so the partition-dim reduction is a fused ones-column in the
att@v matmul.

Phases A (qkT) and B (v) are fused into 4 column passes that stream the x
chunks as their DMAs land (x is split across 3 DMA queues by partition
range; weights ride the gpsimd queue chunk-by-chunk), keeping the PE fed
during the load window.

Head pairs (2h, 2h+1) sit at partition bases 0/64 of shared tiles, so score
matmuls for a pair occupy disjoint PE row groups (concurrent when adjacent),
and the projection contracts K=128 across a pair in one matmul.

dtype variants: "f32" (exact), "f32r" (tf32-like PE fast path), "bf16".
"""

import numpy as np

B, T, C = 2, 2048, 1024
N_HEAD = 16
MEMORY = 256
D = 64           # head dim
G = 4            # head groups (tensor parallel)
HPG = 4          # heads per group
GC = HPG * D     # 256 columns per group
N_CORES = 8
TB = T // 128    # 16 row blocks
SB = T // 256    # 8 query super-blocks

_PROGRAM_CACHE = {}


def _emit(tc, nc, xT, wqkv, wp, ones_in, out, dtype):
    import concourse.mybir as mybir

    f32 = mybir.dt.float32
    mmdt = {
        "f32": f32,
        "f32r": mybir.dt.float32r,
        "bf16": mybir.dt.bfloat16,
    }[dtype]
    # dtype of the yT/sums accumulator tile (needs f32-ish for reciprocal)
    ytdt = f32 if dtype == "bf16" else mmdt
    pool_masks = dtype != "f32r"   # affine_select can't write f32r

    from contextlib import ExitStack

    ctx = ExitStack()
    with ctx:
        const = ctx.enter_context(tc.tile_pool(name="const", bufs=1))
        wpool = ctx.enter_context(tc.tile_pool(name="wpool", bufs=1))
        arena = ctx.enter_context(tc.tile_pool(name="arena", bufs=9))
        qkt_pool = ctx.enter_context(tc.tile_pool(name="qkt", bufs=1))
        vplus_pool = ctx.enter_context(tc.tile_pool(name="vplus", bufs=1))
        expst_pool = ctx.enter_context(tc.tile_pool(name="expst", bufs=4))
        outsb_pool = ctx.enter_context(tc.tile_pool(name="outsb", bufs=4))
        ps1 = ctx.enter_context(tc.tile_pool(name="ps1", bufs=2, space="PSUM"))
        ps2 = ctx.enter_context(tc.tile_pool(name="ps2", bufs=3, space="PSUM"))

        # ---- constants / masks ----
        if not pool_masks:
            from concourse.masks import (
                make_lower_triangular,
                make_upper_triangular,
            )

            up_mask = const.tile([128, 128], f32, name="up_mask", tag="up_mask")
            make_upper_triangular(nc, up_mask[:], val=1.0, diag=True)  # 1 if p<=f
            lo_mask = const.tile([128, 128], f32, name="lo_mask", tag="lo_mask")
            make_lower_triangular(nc, lo_mask[:], val=1.0, diag=True)  # 1 if p>=f
        else:
            up_mask = lo_mask = None

        def mask_up(sl):
            # keep p <= f, else 0   (sl is a [128, 128] slice of expst)
            if pool_masks:
                nc.gpsimd.affine_select(
                    out=sl, in_=sl, compare_op=mybir.AluOpType.is_ge,
                    fill=0.0, base=0, pattern=[[1, 128]], channel_multiplier=-1,
                )
            else:
                nc.vector.tensor_mul(sl, sl, up_mask[:])

        def mask_lo(sl):
            # keep p >= f, else 0
            if pool_masks:
                nc.gpsimd.affine_select(
                    out=sl, in_=sl, compare_op=mybir.AluOpType.is_ge,
                    fill=0.0, base=0, pattern=[[-1, 128]], channel_multiplier=1,
                )
            else:
                nc.vector.tensor_mul(sl, sl, lo_mask[:])

        # ones are DMA'd from DRAM (memset can't produce f32r)
        ones_sb = const.tile([128, 64], mmdt, name="ones_sb", tag="ones_sb")

        # two side-by-side lower-triangular keep masks (for the DVE mask path)
        lo2_view = None
        if pool_masks:
            lo2 = const.tile([128, 256], mmdt, name="lo2", tag="lo2")
            lo2_view = lo2.rearrange("p (b j) -> p b j", b=2, j=128)
            nc.vector.memset(lo2[:], 1.0)
            nc.gpsimd.affine_select(
                out=lo2_view, in_=lo2_view,
                compare_op=mybir.AluOpType.is_ge, fill=0.0,
                base=0, pattern=[[0, 2], [-1, 128]], channel_multiplier=1,
            )

        # ---- input tiles ----
        xT_sb, wqkv_sb = [], []
        for k in range(8):
            xT_sb.append(arena.tile([128, T], mmdt, name=f"xT{k}", tag="arena"))
            wqkv_sb.append(wpool.tile([128, 3 * GC], mmdt, name=f"wqkv{k}",
                                      tag=f"wqkv{k}"))
        wqk_sb = [t[:, 0:2 * GC] for t in wqkv_sb]
        wv_sb = [t[:, 2 * GC:3 * GC] for t in wqkv_sb]
        # v for all 16 row blocks lives in one tile; col 64 of the last dim is
        # the fused ones column (softmax denominator accumulator)
        vp = vplus_pool.tile([128, TB, HPG, D + 1], mmdt, name="vplus",
                             tag="vplus")
        if dtype == "f32r":
            # memset can't write f32r; fall back to DMA'ing the ones column
            pass
        else:
            nc.vector.memset(vp[:, :, :, D:D + 1], 1.0)

        # ---- input DMAs: whole chunks alternate between the two HW DGE
        # queues (one shared DGE processor serves all queues; big descriptors
        # on 2 queues empirically hit ~230GB/s, finer splits throttle) ----
        for k in range(8):
            qa = nc.sync if k % 2 == 0 else nc.scalar
            qb = nc.scalar if k % 2 == 0 else nc.sync
            qa.dma_start(xT_sb[k][:], xT[k * 128:(k + 1) * 128, :])
            qb.dma_start(wqkv_sb[k][:], wqkv[k * 128:(k + 1) * 128, :])
        wp_sb = []
        for pr in range(2):
            t = wpool.tile([128, C], mmdt, name=f"wp{pr}", tag=f"wp{pr}")
            nc.gpsimd.dma_start(t[:], wp[pr * 128:(pr + 1) * 128, :])
            wp_sb.append(t)
        nc.sync.dma_start(ones_sb[:], ones_in[:, 0:64])
        if dtype == "f32r":
            for tb in range(TB):
                nc.sync.dma_start(
                    vp[:, tb, :, D:D + 1],
                    ones_in[:, 64:64 + HPG].rearrange("p (h o) -> p h o", o=1),
                )

        # ---- phases A+B fused: 4 column passes streaming the x chunks ----
        # pass p: A columns t4=p (qkT[:, p*512:(p+1)*512], 4 accum chains) and
        # B row blocks (passes 0/1 each cover 8 of the 16 tb chains).
        qkT_sb = []
        for m in range(4):
            qkT_sb.append(qkt_pool.tile([128, T], mmdt, name=f"qkT{m}",
                                        tag=f"qkT{m}"))
        for t4 in range(4):
            # one accumulation chain per PSUM bank: 4 A chains (qkT row
            # blocks) + 4 B chains (v row blocks), streaming the x chunks
            pa = [ps2.tile([128, 1024], f32, name="psA", tag="st")
                  for _ in range(2)]
            psA = [pa[m // 2][:, (m % 2) * 512:(m % 2 + 1) * 512]
                   for m in range(4)]
            pb = [ps1.tile([128, 512], f32, name="psB", tag="ps1")
                  for _ in range(2)]
            pbx = ps2.tile([128, 1024], f32, name="psBx", tag="st")
            psB = [t[:, 0:256] for t in pb]
            psB += [pbx[:, 0:256], pbx[:, 512:768]]
            tbs = list(range(t4 * 4, t4 * 4 + 4))
            for k in range(8):
                for m in range(4):
                    nc.tensor.matmul(
                        psA[m],
                        wqk_sb[k][:, m * 128:(m + 1) * 128],
                        xT_sb[k][:, t4 * 512:(t4 + 1) * 512],
                        start=(k == 0),
                        stop=(k == 7),
                    )
                for i, tb in enumerate(tbs):
                    nc.tensor.matmul(
                        psB[i],
                        xT_sb[k][:, tb * 128:(tb + 1) * 128],
                        wv_sb[k][:],
                        start=(k == 0),
                        stop=(k == 7),
                    )
            for m in range(4):
                nc.scalar.copy(qkT_sb[m][:, t4 * 512:(t4 + 1) * 512], psA[m])
            for i, tb in enumerate(tbs):
                nc.vector.tensor_copy(
                    vp[:, tb, :, 0:D],
                    psB[i].rearrange("p (h d) -> p h d", h=HPG),
                )

        # per-head views into qkT: q rows = h*64.., k rows = 256 + h*64..
        def qT_h(h):
            return qkT_sb[h // 2][(h % 2) * 64:(h % 2) * 64 + 64, :]

        def kT_h(h):
            return qkT_sb[2 + h // 2][(h % 2) * 64:(h % 2) * 64 + 64, :]

        # ---- phases C+D per head pair ----
        # yt_h: rows 0..63 = y.T (unnormalized), row 64 = softmax denominators
        # ytn_pair[pr]: normalized y.T for heads (2pr, 2pr+1) at bases 0/64
        yt_sb = [None] * HPG
        ytn_sb = []
        for pr in range(2):
            t = arena.tile([128, T], mmdt, name=f"ytn{pr}", tag=f"ytn{pr}", bufs=1)
            ytn_sb.append(t)
        rt_sb = [
            const.tile([32, 128], ytdt, name=f"rt{p}", tag=f"rt{p}")
            for p in range(2)
        ]
        if dtype == "bf16":
            # pair-stacked unnormalized y (bases 0/64), denom rows separate
            ytp_sb = [
                arena.tile([128, T], f32, name=f"ytp{p}", tag=f"ytp{p}",
                           bufs=1)
                for p in range(2)
            ]
            dn_sb = [
                [const.tile([1, T], f32, name=f"dn{p}_{hh}", tag=f"dn{p}_{hh}")
                 for hh in range(2)]
                for p in range(2)
            ]
            rrp_sb = [
                const.tile([2, T], mmdt, name=f"rrp{p}", tag=f"rrp{p}")
                for p in range(2)
            ]
            rtb_sb = [
                const.tile([32, 128], mmdt, name=f"rtb{p}", tag=f"rtb{p}")
                for p in range(2)
            ]
            # K=2 indicator: row 0 -> out partitions 0:64, row 1 -> 64:128
            # (built with two affine_selects; sub-partition-aligned memsets
            # are rejected by the BIR verifier)
            ind2 = const.tile([2, 128], mmdt, name="ind2", tag="ind2")
            nc.vector.memset(ind2[:], 1.0)
            nc.gpsimd.affine_select(
                out=ind2[:], in_=ind2[:],
                compare_op=mybir.AluOpType.is_ge, fill=0.0,
                base=0, pattern=[[1, 128]], channel_multiplier=-64,
            )
            nc.gpsimd.affine_select(
                out=ind2[:], in_=ind2[:],
                compare_op=mybir.AluOpType.is_ge, fill=0.0,
                base=63, pattern=[[-1, 128]], channel_multiplier=64,
            )

        def roles_for(sb):
            roles = []
            for dtk in (-2, -1, 0, 1):
                tkb = 2 * sb + dtk
                if 0 <= tkb:
                    roles.append((tkb, "abcd"[dtk + 2]))
            return roles

        # C score layout per head, 768 cols of the st psum / expst tile:
        #   a @ [0:128)    key blk 2sb-2, queries 0:128 of sb, lower-tri keep
        #   b @ [128:384)  key blk 2sb-1, all queries; right half lower-tri
        #   c @ [384:640)  key blk 2sb,   all queries; left half upper-tri
        #   d @ [640:768)  key blk 2sb+1, queries 128:256, upper-tri keep
        # The two lower-tri 128-blocks sit at offsets {0, 256} and the two
        # upper-tri at {384, 640}, so one 4D-strided affine_select per
        # triangle type masks both heads of the pair at once.
        def emit_C(pr):
            heads = (2 * pr, 2 * pr + 1)
            if dtype != "bf16":
                for h in heads:
                    yt_sb[h] = arena.tile([65, T], ytdt, name=f"yt{h}",
                                          tag="arena")
            for sb in range(SB):
                roles = roles_for(sb)
                n = len(roles)
                # st psum: group (a,b) in bank 0, group (c,d) in bank 1
                # (matmul start=True resets the whole addressed bank)
                off = {"a": (0, 128), "b": (128, 384), "c": (512, 768),
                       "d": (768, 896)}
                # expst stays compact: a@0, b@128, c@384, d@640
                eoff = {"a": (0, 128), "b": (128, 384), "c": (384, 640),
                        "d": (640, 768)}
                qoff = {"a": (0, 128), "b": (0, 256), "c": (0, 256),
                        "d": (128, 256)}
                # scores for the pair, interleaved: disjoint PE row groups
                # (bases 0/64) run concurrently when adjacent in the queue
                st = {}
                for h in heads:
                    st[h] = ps2.tile([128, 1024], f32, name=f"st{h % 2}",
                                     tag="st")
                # per-head back-to-back chains pipeline with ~zero overhead
                # (the PE preloads the next weights while streaming)
                for h in heads:
                    for i, (tkb, role) in enumerate(roles):
                        c0, c1 = off[role]
                        q0, q1 = qoff[role]
                        nc.tensor.matmul(
                            st[h][:, c0:c1],
                            kT_h(h)[:, tkb * 128:(tkb + 1) * 128],
                            qT_h(h)[:, sb * 256 + q0:sb * 256 + q1],
                            start=(i % 2 == 0),
                            stop=(i % 2 == 1 or i == n - 1),
                        )
                ep = expst_pool.tile([128, 2, 768], mmdt, name="expst",
                                     tag="expst")
                for hh, h in enumerate(heads):
                    if n == 4:
                        nc.scalar.activation(
                            ep[:, hh, 0:384],
                            st[h][:, 0:384],
                            mybir.ActivationFunctionType.Exp,
                            scale=0.125,
                        )
                    nc.scalar.activation(
                        ep[:, hh, 384:768],
                        st[h][:, 512:896],
                        mybir.ActivationFunctionType.Exp,
                        scale=0.125,
                    )
                if pool_masks:
                    for hh in range(2):
                        if n == 4:
                            # lower-tri keep on blocks a@0, b_right@256:
                            # DVE multiply against the two-triangle tile
                            dv = ep[:, hh, 0:512].rearrange(
                                "p (x j) -> p x j", x=2, j=256)[:, :, 0:128]
                            nc.vector.tensor_mul(dv, dv, lo2_view)
                        # upper-tri keep on blocks c_left@384, d@640
                        uv = ep[:, hh, 256:768].rearrange(
                            "p (x j) -> p x j", x=2, j=256)[:, :, 128:256]
                        nc.gpsimd.affine_select(
                            out=uv, in_=uv,
                            compare_op=mybir.AluOpType.is_ge, fill=0.0,
                            base=0, pattern=[[0, 2], [1, 128]],
                            channel_multiplier=-1,
                        )
                else:
                    for hh in range(2):
                        if n == 4:
                            nc.vector.tensor_mul(
                                ep[:, hh, 0:128], ep[:, hh, 0:128], lo_mask[:])
                            nc.vector.tensor_mul(
                                ep[:, hh, 256:384], ep[:, hh, 256:384],
                                lo_mask[:])
                        nc.vector.tensor_mul(
                            ep[:, hh, 384:512], ep[:, hh, 384:512], up_mask[:])
                        nc.vector.tensor_mul(
                            ep[:, hh, 640:768], ep[:, hh, 640:768], up_mask[:])
                for hh, h in enumerate(heads):
                    yts = ps1.tile([65, 256], f32, name="yts", tag="ps1")
                    order = [r for r in roles if r[1] in "bc"] + [
                        r for r in roles if r[1] in "ad"
                    ]
                    for j, (tkb, role) in enumerate(order):
                        c0, c1 = eoff[role]
                        q0, q1 = qoff[role]
                        nc.tensor.matmul(
                            yts[:, q0:q1],
                            vp[:, tkb, h, :],
                            ep[:, hh, c0:c1],
                            start=(j == 0),
                            stop=(j == n - 1),
                        )
                    if dtype == "bf16":
                        r0 = (h % 2) * 64
                        nc.vector.tensor_copy(
                            ytp_sb[pr][r0:r0 + 64, sb * 256:(sb + 1) * 256],
                            yts[0:64, :],
                        )
                        nc.vector.tensor_copy(
                            dn_sb[pr][h % 2][:, sb * 256:(sb + 1) * 256],
                            yts[64:65, :],
                        )
                    else:
                        nc.vector.tensor_copy(
                            yt_sb[h][:, sb * 256:(sb + 1) * 256], yts[:]
                        )

        def emit_D_recip(pr):
            # reciprocal on [1, 2048] is ~13us on one DVE lane; bounce the
            # rows through a [32, 128] tile with tiny SBUF->SBUF DMAs instead
            heads = (2 * pr, 2 * pr + 1)
            rt = rt_sb[pr]
            for h in heads:
                r0 = (h % 2) * 16
                if dtype == "bf16":
                    nc.sync.dma_start(rt[r0:r0 + 16, :], dn_sb[pr][h % 2][:])
                else:
                    nc.sync.dma_start(rt[r0:r0 + 16, :], yt_sb[h][64:65, :])
            with nc.allow_low_precision(reason="softmax denom reciprocal"):
                if dtype == "bf16":
                    nc.vector.reciprocal(rtb_sb[pr][0:32, :], rt[0:32, :])
                else:
                    nc.vector.reciprocal(rt[0:32, :], rt[0:32, :])
            if dtype == "bf16":
                # [32,128] rows map linearly onto [2, T]: one DMA fills both
                # heads' reciprocal rows
                nc.sync.dma_start(rrp_sb[pr][0:2, :], rtb_sb[pr][0:32, :])
            else:
                for h in heads:
                    r0 = (h % 2) * 16
                    nc.sync.dma_start(yt_sb[h][64:65, :], rt[r0:r0 + 16, :])

        def emit_D_norm(pr, t4s):
            heads = (2 * pr, 2 * pr + 1)
            for t4 in t4s:
                sl = slice(t4 * 512, (t4 + 1) * 512)
                if dtype == "bf16":
                    # pair-stacked broadcast: one K=2 matmul + one [128,512]
                    # DVE multiply normalize both heads of the pair
                    bc = ps1.tile([128, 512], f32, name="bc", tag="ps1")
                    nc.tensor.matmul(
                        bc[:], ind2[:, :], rrp_sb[pr][:, sl],
                        start=True, stop=True,
                    )
                    nc.vector.tensor_mul(
                        ytn_sb[pr][:, sl], ytp_sb[pr][:, sl], bc[:],
                    )
                    continue
                for h in heads:
                    rrow = yt_sb[h][64:65, :]
                    ones_sl = ones_sb[64:65, :]
                    bc = ps1.tile([64, 512], f32, name="bc", tag="ps1")
                    nc.tensor.matmul(
                        bc[:], ones_sl, rrow[0:1, sl],
                        start=True, stop=True,
                    )
                    nc.vector.tensor_mul(
                        ytn_sb[pr][(h % 2) * 64:(h % 2) * 64 + 64, sl],
                        yt_sb[h][0:64, sl],
                        bc[:],
                    )

        # ---- phase E: partial projection out = y_g @ W_p[g] (K=128 pairs) --
        outdt = mmdt if dtype == "bf16" else f32
        store_q = [nc.sync, nc.scalar]

        def emit_E(tbs):
            for tb in tbs:
                for nh in range(2):
                    ps = ps2.tile([128, 512], f32, name="psE", tag="st")
                    for pr in range(2):
                        nc.tensor.matmul(
                            ps[:],
                            ytn_sb[pr][:, tb * 128:(tb + 1) * 128],
                            wp_sb[pr][:, nh * 512:(nh + 1) * 512],
                            start=(pr == 0),
                            stop=(pr == 1),
                        )
                    ob = outsb_pool.tile([128, 512], outdt, name="outsb",
                                         tag="outsb")
                    if (tb + nh) % 2 == 0:
                        nc.scalar.copy(ob[:], ps[:])
                    else:
                        nc.vector.tensor_copy(ob[:], ps[:])
                    qo = store_q[(tb * 2 + nh) % 2]
                    qo.dma_start(
                        out[tb * 128:(tb + 1) * 128,
                            nh * 512:(nh + 1) * 512], ob[:]
                    )

        emit_C(0)
        emit_D_recip(0)
        emit_C(1)
        if dtype == "bf16":
            emit_D_recip(1)
            # interleave normalization with the projection per column block
            for t4 in range(4):
                emit_D_norm(0, [t4])
                emit_D_norm(1, [t4])
                emit_E(range(t4 * 4, t4 * 4 + 4))
        else:
            emit_D_norm(0, range(4))
            emit_D_recip(1)
            emit_D_norm(1, range(4))
            emit_E(range(TB))


def build_program(dtype="bf16"):
    key = ("v14", dtype)
    if key in _PROGRAM_CACHE:
        return _PROGRAM_CACHE[key]
    import concourse.bacc as bacc
    import concourse.mybir as mybir
    import concourse.tile as tile

    f32 = mybir.dt.float32
    mmdt = {
        "f32": f32,
        "f32r": mybir.dt.float32r,
        "bf16": mybir.dt.bfloat16,
    }[dtype]
    outdt = mmdt if dtype == "bf16" else f32
    nc = bacc.Bacc("TRN2", target_bir_lowering=False, debug=False, num_devices=N_CORES)
    xT = nc.dram_tensor("xT", [C, T], mmdt, kind="ExternalInput").ap()
    wqkv = nc.dram_tensor("wqkv", [C, 3 * GC], mmdt, kind="ExternalInput").ap()
    wp = nc.dram_tensor("wp", [GC, C], mmdt, kind="ExternalInput").ap()
    ones_in = nc.dram_tensor("ones_in", [128, 64 + HPG], mmdt,
                             kind="ExternalInput").ap()
    out = nc.dram_tensor("out", [T, C], outdt, kind="ExternalOutput").ap()
    with tile.TileContext(nc) as tc:
        _emit(tc, nc, xT, wqkv, wp, ones_in, out, dtype)
    nc.compile()
    _PROGRAM_CACHE[key] = nc
    return nc


def make_in_maps(x, W_attn, W_proj, dtype="bf16"):
    x = np.asarray(x, dtype=np.float32)
    W_attn = np.asarray(W_attn, dtype=np.float32)
    W_proj = np.asarray(W_proj, dtype=np.float32)
    if dtype == "bf16":
        import ml_dtypes

        cast = lambda a: np.ascontiguousarray(a, dtype=ml_dtypes.bfloat16)
    else:
        cast = lambda a: np.ascontiguousarray(a, dtype=np.float32)
    xTs = [cast(x[b].T) for b in range(B)]
    in_maps = []
    for c in range(N_CORES):
        b, g = divmod(c, G)
        q_cols = W_attn[:, g * GC:(g + 1) * GC]
        k_cols = W_attn[:, C + g * GC:C + (g + 1) * GC]
        v_cols = W_attn[:, 2 * C + g * GC:2 * C + (g + 1) * GC]
        in_maps.append({
            "xT": xTs[b],
            "wqkv": cast(np.concatenate([q_cols, k_cols, v_cols], axis=1)),
            "wp": cast(W_proj[g * GC:(g + 1) * GC, :]),
            "ones_in": cast(np.ones((128, 64 + HPG), dtype=np.float32)),
        })
    return in_maps


def gather(results):
    out = np.zeros((B, T, C), dtype=np.float32)
    for c, res in enumerate(results):
        b = c // G
        out[b] += np.asarray(res["out"], dtype=np.float32)
    return out


def kernel(x, W_attn, W_proj, dtype="bf16", trace=False):
    from concourse import bass_utils

    nc = build_program(dtype=dtype)
    in_maps = make_in_maps(x, W_attn, W_proj, dtype=dtype)
    r = bass_utils.run_bass_kernel_spmd(
        nc, in_maps, core_ids=list(range(N_CORES)), trace=trace
    )
    out = gather(r.results)
    if trace:
        kernel.last_results = r
    return out

